# revision 64
# baseline (speedup 1.0000x reference)
"""Causal multi-head self-attention with RoPE on 8 Trainium2 NeuronCores.

Sharding: tensor-parallel over heads — core c owns heads (2c, 2c+1) for BOTH
batch elements.  Feature dim lives on partitions, tokens on the free dim.

  phase A  (software-pipelined per 512-token chunk, per batch)
           qT/kT/vT = W @ x^T (f16 matmuls, K=1024 contraction);
           RoPE on qT/kT with HOST-precomputed cos/sin f16 tables
           (rot = x*C + swap(x)*S with the sign folded into S);
           vT transposed to (token, dim) 130-col k-tile layout with a ones
           column per head (denominator comes free out of the AV matmul).
  phase B  per (batch, q-chunk 512) — emitted with a 1-chunk skew so chunk
           t+1's projections hide chunk t's RoPE latency:
             logitsT (k-part, q-free) f16 = kT_h^T @ qT_h, 2 heads packed in
             one [128, 1024] psum; e = exp(logits/8) -> f16;
             AV runs transposed: ctx[q-part, 65] += e_blk^T @ [v | 1], one
             65-wide matmul per (head, q-tile 128, k-tile) — only q-tiles on
             or below the diagonal. Diagonal k-tiles get one 128x128
             triangular mask multiply per head.
           normalize by the ones-column (per-partition scalar on Pool),
           PE-transpose back to (dim, token), stage as f16.
  phase C  batch 0: one 8-core AllToAll (512 KB) exchanging half-chunks
           (core d gets tokens qc*512 + half*256 with qc = d//2, half =
           d%2), fully hidden under batch 1's compute.  Batch 1: TWO
           quarter-chunk AllToAlls (256 KB each) — the first (q-chunks 0-1)
           fires mid-attention and hides as well; only the second (q-chunks
           2-3, ~21 us) plus one 128-token projection sits in the serial
           tail.  Local out-projection with wo^T per arrival; dummy PE
           transposes keep the p-state warm across the last collective.
           Each core returns (2, 256, 1024); the host reassembles (batch 1
           uses the quarter-chunk slot mapping).
"""
import os
import sys

import numpy as np

for p in ("/opt/trn_rl_repo", "/root/.axon_site/_ro/trn_rl_repo"):
    if os.path.isdir(p) and p not in sys.path:
        sys.path.insert(0, p)

D_MODEL = 1024
NUM_HEADS = 16
D_K = 64
THETA = 10000.0
BATCH = 2
SEQ = 2048
NCORES = 8
H_PER_CORE = 2
DIMS = H_PER_CORE * D_K   # 128 ctx dims owned per core
QC = 512                  # q-chunk
KT = 128                  # k-tile
SCALE = 0.125             # 1/sqrt(d_k)

_CACHE = {}


def _build_program():
    import concourse.mybir as mybir
    import concourse.tile as tile
    from concourse import bacc
    from concourse.masks import make_identity, make_upper_triangular

    F32 = mybir.dt.float32
    F16 = mybir.dt.float16
    AFT = mybir.ActivationFunctionType

    nc = bacc.Bacc("TRN2", target_bir_lowering=False, debug=False,
                   num_devices=NCORES)

    xT_d = nc.declare_dram_parameter("xT", [D_MODEL, BATCH * SEQ], F16,
                                     isOutput=False)
    wqT_d = nc.declare_dram_parameter("wqT", [D_MODEL, DIMS], F16, isOutput=False)
    wkT_d = nc.declare_dram_parameter("wkT", [D_MODEL, DIMS], F16, isOutput=False)
    wvT_d = nc.declare_dram_parameter("wvT", [D_MODEL, DIMS], F16, isOutput=False)
    woT_d = nc.declare_dram_parameter("woT", [D_MODEL, D_MODEL], F16, isOutput=False)
    csC_d = nc.declare_dram_parameter("csC", [DIMS, BATCH * SEQ], F16,
                                      isOutput=False)
    csS_d = nc.declare_dram_parameter("csS", [DIMS, BATCH * SEQ], F16,
                                      isOutput=False)
    out_d = nc.declare_dram_parameter("out", [BATCH, 2 * KT, D_MODEL], F32,
                                      isOutput=True)
    DEBUG = bool(os.environ.get("K_DEBUG"))
    if DEBUG:
        dbg_qr = nc.declare_dram_parameter("dbg_qr", [DIMS, SEQ], F16,
                                           isOutput=True)
        dbg_kr = nc.declare_dram_parameter("dbg_kr", [DIMS, SEQ], F16,
                                           isOutput=True)
        dbg_vb = nc.declare_dram_parameter("dbg_vb", [128, 130 * (SEQ // KT)],
                                           F16, isOutput=True)
        dbg_st = nc.declare_dram_parameter("dbg_st", [NCORES, DIMS, 2 * KT],
                                           F16, isOutput=True)
        dbg_px = nc.declare_dram_parameter("dbg_px", [128, QC], F32,
                                           isOutput=True)
        dbg_et = nc.declare_dram_parameter("dbg_et", [2, 128, 2 * QC], F32,
                                           isOutput=True)

    NCH = SEQ // QC           # 4 chunks per batch
    NVT = QC // KT            # 4 k-tiles per chunk

    with tile.TileContext(nc) as tc:
        with tc.tile_pool(name="consts", bufs=1) as consts, \
             tc.tile_pool(name="qk", bufs=1) as qkp, \
             tc.tile_pool(name="vbufp", bufs=1) as vbufp, \
             tc.tile_pool(name="ps", bufs=1, space="PSUM") as ps, \
             tc.tile_pool(name="epool", bufs=17) as epool, \
             tc.tile_pool(name="rawp", bufs=3) as rawp, \
             tc.tile_pool(name="xtp", bufs=1) as xtp, \
             tc.tile_pool(name="ropep", bufs=3) as ropep, \
             tc.tile_pool(name="normp", bufs=6) as normp, \
             tc.tile_pool(name="stp", bufs=3) as stp, \
             tc.tile_pool(name="wp", bufs=1) as wp, \
             tc.tile_pool(name="outp", bufs=2) as outp, \
             tc.tile_pool(name="dram", bufs=1, space="DRAM") as dram:

            # ---------- constants ----------
            tri_f = consts.tile([KT, KT], F32)
            make_upper_triangular(nc, tri_f[:], val=1.0, diag=True)
            tri = consts.tile([KT, KT], F16)
            nc.vector.tensor_copy(tri, tri_f)
            ident = consts.tile([128, 128], F32)
            make_identity(nc, ident[:])
            ones16 = consts.tile([128, 16], F16)
            nc.vector.memset(ones16, 1.0)

            # cos/sin tables for both batches (host-precomputed); loaded
            # after the first x chunk (see below)
            csC = consts.tile([DIMS, BATCH * SEQ], F16, name="csC")
            csS = consts.tile([DIMS, BATCH * SEQ], F16, name="csS")

            a2a_in = [dram.tile([NCORES, DIMS, 2 * KT], F16, name=f"a2ain{b}")
                      for b in range(BATCH)]
            a2a_out = [dram.tile([NCORES, DIMS, 2 * KT], F16, name=f"a2aout{b}")
                       for b in range(BATCH)]
            a2a1_in = [dram.tile([NCORES, DIMS, KT], F16, name=f"a2b1in{h}")
                       for h in range(2)]
            a2a1_out = [dram.tile([NCORES, DIMS, KT], F16, name=f"a2b1out{h}")
                        for h in range(2)]

            # per-batch persistent tiles
            qR = {b: qkp.tile([DIMS, SEQ], F16, tag=f"qR{b}", name=f"qR{b}")
                  for b in range(BATCH)}
            kR = {b: qkp.tile([DIMS, SEQ], F16, tag=f"kR{b}", name=f"kR{b}")
                  for b in range(BATCH)}
            vbuf = {b: vbufp.tile([128, 130 * (SEQ // KT)], F16, tag=f"vb{b}",
                                  name=f"vbuf{b}")
                    for b in range(BATCH)}

            # projection weights
            w_sb = {}
            for nm, d in (("q", wqT_d), ("k", wkT_d), ("v", wvT_d)):
                wt = wp.tile([128, 8, DIMS], F16, tag=f"w{nm}", name=f"w{nm}")
                nc.sync.dma_start(
                    out=wt, in_=d[:].rearrange("(e p) c -> p e c", p=128))
                w_sb[nm] = [wt[:, k8, :] for k8 in range(8)]

            def emit_wo_loads():
                wo_sb = []
                for k8 in range(8):
                    t = wp.tile([128, D_MODEL], F16, tag=f"wo{k8}",
                                name=f"wo{k8}")
                    nc.sync.dma_start(out=t,
                                      in_=woT_d[k8 * 128:(k8 + 1) * 128, :])
                    wo_sb.append(t)
                return wo_sb

            # whole-x residency: 8 tiles of [128, 4096] f16, loaded in
            # dependency-epoch order so chunk 0 can start at once
            xt_all = [xtp.tile([128, BATCH * SEQ], F16, tag=f"xt{k8}",
                               name=f"xt{k8}")
                      for k8 in range(8)]

            def load_x(c0, c1):
                for k8 in range(8):
                    nc.sync.dma_start(
                        out=xt_all[k8][:, c0:c1],
                        in_=xT_d[k8 * 128:(k8 + 1) * 128, c0:c1])

            load_x(0, QC)
            nc.sync.dma_start(out=csC[:, 0:QC], in_=csC_d[:, 0:QC])
            nc.sync.dma_start(out=csS[:, 0:QC], in_=csS_d[:, 0:QC])
            load_x(QC, 2 * QC)
            nc.sync.dma_start(out=csC[:, QC:], in_=csC_d[:, QC:])
            nc.sync.dma_start(out=csS[:, QC:], in_=csS_d[:, QC:])
            load_x(2 * QC, SEQ)
            load_x(SEQ, SEQ + QC)
            load_x(SEQ + QC, 2 * SEQ)

            # ---------- phase A: one 512-token chunk ----------
            def emit_chunk(b, t):
                g0 = b * SEQ + t * QC          # global column offset
                c0 = t * QC                    # within-batch column offset
                # q,k projections -> one (dim, 2*QC) tile [q | k]
                rqk = rawp.tile([DIMS, 2 * QC], F16, tag="rawqk", name="rawqk")
                for ni, nm in enumerate(("q", "k")):
                    pp = ps.tile([128, QC], F32, tag="m1", bufs=2, name="pp")
                    for k8 in range(8):
                        nc.tensor.matmul(pp, w_sb[nm][k8],
                                         xt_all[k8][:, g0:g0 + QC],
                                         start=(k8 == 0), stop=(k8 == 7))
                    nc.vector.tensor_copy(rqk[:, ni * QC:(ni + 1) * QC], pp)

                # v: projected directly transposed, (token, dim) per 128-tile
                pv = ps.tile([128, QC], F32, tag="m1", bufs=2, name="pv")
                for i in range(NVT):
                    for k8 in range(8):
                        nc.tensor.matmul(
                            pv[:, i * KT:(i + 1) * KT],
                            xt_all[k8][:, g0 + i * KT:g0 + (i + 1) * KT],
                            w_sb["v"][k8],
                            start=(k8 == 0), stop=(k8 == 7),
                            skip_group_check=True)
                vb = vbuf[b]
                cview = vb[:, 130 * NVT * t:130 * NVT * (t + 1)]
                v_view = cview.rearrange("p (n c) -> p n c", c=130)
                for col in (64, 129):
                    nc.vector.tensor_copy(
                        v_view[:, :, col:col + 1].rearrange("p n c -> p (n c)"),
                        ones16[:, 0:NVT])
                dst = cview.rearrange("p (n h c) -> p n h c", h=2, c=65)[
                    :, :, :, 0:64]
                src = pv[:].rearrange("p (n h c) -> p n h c", h=2, c=64)
                nc.vector.tensor_copy(dst, src)

                # RoPE: rot = x*C + swap(x)*S   (sign baked into S rows)
                src = rqk[:].rearrange("(p two) n -> p two n", two=2)
                swp = ropep.tile([DIMS, 2 * QC], F16, tag="swap", name="swp")
                dstv = swp[:].rearrange("(p two) n -> p two n", two=2)
                nc.gpsimd.dma_start(out=dstv[:, 0, :], in_=src[:, 1, :])
                nc.gpsimd.dma_start(out=dstv[:, 1, :], in_=src[:, 0, :])
                for ni, nm in enumerate(("q", "k")):
                    nsl = slice(ni * QC, (ni + 1) * QC)
                    t1 = ropep.tile([DIMS, QC], F16, tag="t1", name="t1")
                    nc.vector.tensor_mul(t1, rqk[:, nsl], csC[:, g0:g0 + QC])
                    nc.vector.tensor_mul(swp[:, nsl], swp[:, nsl],
                                         csS[:, g0:g0 + QC])
                    dst = (qR if nm == "q" else kR)[b][:, c0:c0 + QC]
                    nc.vector.tensor_add(dst, t1, swp[:, nsl])

            # ---------- phase B: one q-chunk of attention ----------
            def emit_attn(b, qc):
                pctx = [ps.tile([128, QC], F32, tag=f"ctx{h}", bufs=1,
                                name=f"pctx{h}")
                        for h in range(H_PER_CORE)]
                nkt = NVT * qc + NVT
                ets = []
                for kt in range(nkt):
                    j = kt - NVT * qc            # >=0: diagonal band tile
                    q0 = 0 if j < 0 else KT * j
                    pl = ps.tile([128, 2 * QC], F32, tag="logit", bufs=2,
                                 name="pl")
                    for h in range(H_PER_CORE):
                        nc.tensor.matmul(
                            pl[:, h * QC + q0:(h + 1) * QC],
                            kR[b][64 * h:64 * (h + 1), kt * KT:(kt + 1) * KT],
                            qR[b][64 * h:64 * (h + 1),
                                  qc * QC + q0:(qc + 1) * QC],
                            start=True, stop=True)
                    et = epool.tile([128, 2 * QC], F16, tag="e", name="et")
                    if q0 == 0:
                        nc.scalar.activation(et, pl, AFT.Exp, scale=SCALE)
                    else:
                        ev = et[:].rearrange("p (h n) -> p h n", h=2)[
                            :, :, q0:QC]
                        pv = pl[:].rearrange("p (h n) -> p h n", h=2)[
                            :, :, q0:QC]
                        nc.scalar.activation(ev, pv, AFT.Exp, scale=SCALE)
                    if j >= 0:
                        for h in range(H_PER_CORE):
                            msl = slice(h * QC + q0, h * QC + q0 + KT)
                            nc.vector.tensor_mul(et[:, msl], et[:, msl], tri)
                    if DEBUG and b == 0 and qc == 0 and kt <= 1:
                        dt_ = normp.tile([128, 2 * QC], F32, tag="dbge",
                                         name="dbge", bufs=1)
                        nc.vector.tensor_copy(dt_, et)
                        nc.sync.dma_start(out=dbg_et[kt], in_=dt_)
                    ets.append(et)
                # AV, one accumulation group per (h, q-tile): only one open
                # group per psum bank at a time (PE constraint)
                for qt in range(NVT):
                    for kt in range(NVT * qc + qt + 1):
                        for h in range(H_PER_CORE):
                            vt = vbuf[b][:, 130 * kt + 65 * h:
                                         130 * kt + 65 * h + 65]
                            nc.tensor.matmul(
                                pctx[h][:, qt * KT:qt * KT + 65],
                                ets[kt][:, h * QC + qt * KT:
                                        h * QC + (qt + 1) * KT],
                                vt,
                                start=(kt == 0),
                                stop=(kt == NVT * qc + qt),
                                skip_group_check=True)

                if DEBUG and b == 0 and qc == 0:
                    dpx = normp.tile([128, QC], F32, tag="dbgp", name="dbgp",
                                     bufs=1)
                    nc.vector.tensor_copy(dpx, pctx[0][:])
                    nc.sync.dma_start(out=dbg_px[:], in_=dpx)

                # normalize, transpose to (dim, token), stage for the A2A
                recips = []
                for h in range(H_PER_CORE):
                    rc = normp.tile([128, NVT], F32, tag="rc", name="rc")
                    den = pctx[h][:].rearrange("p (n c) -> p n c", c=KT)[
                        :, :, 64:65].rearrange("p n c -> p (n c)")
                    nc.vector.reciprocal(rc, den)
                    recips.append(rc)
                stg = stp.tile([128, QC], F16, tag="stage", name="stage")
                for qt in range(NVT):
                    tp = ps.tile([128, QC], F32, tag="m1", bufs=2, name="tp")
                    nr = normp.tile([128, 128], F32, tag="nr", name="nr")
                    for h in range(H_PER_CORE):
                        nc.vector.tensor_scalar_mul(
                            nr[:, 64 * h:64 * (h + 1)],
                            pctx[h][:, qt * KT:qt * KT + 64],
                            recips[h][:, qt:qt + 1])
                    nc.tensor.transpose(tp[:, 0:128], nr, ident[:])
                    nc.vector.tensor_copy(stg[:, qt * KT:(qt + 1) * KT],
                                          tp[:, 0:128])
                if b == 0:
                    for half in range(2):
                        nc.sync.dma_start(
                            out=a2a_in[0][2 * qc + half],
                            in_=stg[:, half * 2 * KT:(half + 1) * 2 * KT])
                else:
                    # batch 1 exchanges in two quarter-chunk AllToAlls; slot
                    # d of half h = tokens ((2h + qc%2)*512 + (d%4)*128)
                    dst = a2a1_in[qc // 2]
                    for qt in range(NVT):
                        nc.sync.dma_start(
                            out=dst[4 * (qc % 2) + qt],
                            in_=stg[:, qt * KT:(qt + 1) * KT])

            def emit_a2a(b):
                nc.gpsimd.collective_compute(
                    "AllToAll", mybir.AluOpType.bypass,
                    replica_groups=[list(range(NCORES))],
                    ins=[a2a_in[b].opt()], outs=[a2a_out[b].opt()],
                )

            def emit_a2a1(half):
                nc.gpsimd.collective_compute(
                    "AllToAll", mybir.AluOpType.bypass,
                    replica_groups=[list(range(NCORES))],
                    ins=[a2a1_in[half].opt()], outs=[a2a1_out[half].opt()],
                )

            def emit_woproj1(half, wo_sb):
                cm = wp.tile([DIMS, NCORES, KT], F16, tag=f"cm1{half}",
                             name=f"cm1{half}")
                nc.sync.dma_start(
                    out=cm, in_=a2a1_out[half][:].rearrange("e p c -> p e c"))
                ot = outp.tile([128, D_MODEL], F32, tag="out", name="ot")
                for nn in range(2):
                    po = ps.tile([128, QC], F32, tag="m1", bufs=2, name="po")
                    for i in range(NCORES):
                        nc.tensor.matmul(
                            po, cm[:, i, :], wo_sb[i][:, nn * QC:(nn + 1) * QC],
                            start=(i == 0), stop=(i == NCORES - 1))
                    nc.scalar.copy(ot[:, nn * QC:(nn + 1) * QC], po)
                    nc.sync.dma_start(
                        out=out_d[1, half * KT:(half + 1) * KT,
                                  nn * QC:(nn + 1) * QC],
                        in_=ot[:, nn * QC:(nn + 1) * QC])

            def emit_woproj(b, wo_sb):
                cm = wp.tile([DIMS, NCORES, 2 * KT], F16, tag=f"cm{b}",
                             name=f"cm{b}")
                nc.sync.dma_start(
                    out=cm, in_=a2a_out[b][:].rearrange("e p c -> p e c"))
                for mt in range(2):
                    ot = outp.tile([128, D_MODEL], F32, tag="out", name="ot")
                    for nn in range(2):
                        po = ps.tile([128, QC], F32, tag="m1", bufs=2,
                                     name="po")
                        for i in range(NCORES):
                            nc.tensor.matmul(
                                po, cm[:, i, mt * KT:(mt + 1) * KT],
                                wo_sb[i][:, nn * QC:(nn + 1) * QC],
                                start=(i == 0), stop=(i == NCORES - 1))
                        nc.scalar.copy(ot[:, nn * QC:(nn + 1) * QC], po)
                        nc.sync.dma_start(
                            out=out_d[b, mt * KT:(mt + 1) * KT,
                                      nn * QC:(nn + 1) * QC],
                            in_=ot[:, nn * QC:(nn + 1) * QC])

            # ---------- emission schedule ----------
            emit_chunk(0, 0)
            wo_sb = emit_wo_loads()
            emit_chunk(0, 1)
            emit_attn(0, 0)
            emit_chunk(0, 2)
            emit_attn(0, 1)
            emit_chunk(0, 3)
            emit_attn(0, 2)
            emit_chunk(1, 0)
            emit_attn(0, 3)
            emit_chunk(1, 1)
            emit_attn(1, 0)
            emit_chunk(1, 2)
            emit_a2a(0)
            emit_attn(1, 1)
            emit_chunk(1, 3)
            emit_a2a1(0)
            emit_attn(1, 2)
            emit_attn(1, 3)
            emit_a2a1(1)
            emit_woproj(0, wo_sb)
            emit_woproj1(0, wo_sb)
            # keep the PE p-state warm while the last AllToAll is in
            # flight so the final projection runs at full clock
            for _ in range(190):
                dpl = ps.tile([128, QC], F32, tag="m1", bufs=2, name="dpl")
                nc.tensor.transpose(dpl[:, 0:128], ident, ident[:])
            emit_woproj1(1, wo_sb)
            if DEBUG:
                nc.sync.dma_start(out=dbg_qr[:], in_=qR[0])
                nc.sync.dma_start(out=dbg_kr[:], in_=kR[0])
                nc.sync.dma_start(out=dbg_vb[:], in_=vbuf[0])
                cm0 = wp.tile([DIMS, NCORES, 2 * KT], F16, tag="cm0",
                              name="cmdbg")
                nc.sync.dma_start(
                    out=cm0, in_=a2a_out[0][:].rearrange("e p c -> p e c"))
                nc.sync.dma_start(
                    out=dbg_st[:].rearrange("e p c -> p e c"), in_=cm0)

    nc.compile()
    return nc


def _host_prep(inputs):
    x = np.asarray(inputs["in_features"], dtype=np.float32)
    tp = np.asarray(inputs["token_positions"], dtype=np.int32)
    wq = np.asarray(inputs["wq"], dtype=np.float32)
    wk = np.asarray(inputs["wk"], dtype=np.float32)
    wv = np.asarray(inputs["wv"], dtype=np.float32)
    wo = np.asarray(inputs["wo"], dtype=np.float32)

    xT = np.ascontiguousarray(
        np.concatenate([x[b].T for b in range(BATCH)], axis=1)).astype(np.float16)
    woT = np.ascontiguousarray(wo.T).astype(np.float16)

    # cos/sin tables, (dim row, batch*token col); sign baked into S so that
    # rot = x*C + swap(x)*S
    half = D_K // 2
    inv_freq = 1.0 / (THETA ** (2.0 * np.arange(half) / D_K))     # (32,)
    ang = tp.astype(np.float64)[:, :, None] * inv_freq[None, None, :]
    cos = np.cos(ang)                                             # (B, S, 32)
    sin = np.sin(ang)
    rows = np.arange(DIMS)
    j = (rows % D_K) // 2                                         # freq index
    sign = np.where(rows % 2 == 0, -1.0, 1.0)
    csC = np.empty((DIMS, BATCH * SEQ), dtype=np.float16)
    csS = np.empty((DIMS, BATCH * SEQ), dtype=np.float16)
    for b in range(BATCH):
        csC[:, b * SEQ:(b + 1) * SEQ] = cos[b][:, j].T
        csS[:, b * SEQ:(b + 1) * SEQ] = (sin[b][:, j] * sign[None, :]).T

    in_maps = []
    for c in range(NCORES):
        rsl = slice(DIMS * c, DIMS * (c + 1))
        in_maps.append({
            "xT": xT,
            "wqT": np.ascontiguousarray(wq[rsl].T).astype(np.float16),
            "wkT": np.ascontiguousarray(wk[rsl].T).astype(np.float16),
            "wvT": np.ascontiguousarray(wv[rsl].T).astype(np.float16),
            "woT": woT,
            "csC": csC,
            "csS": csS,
        })
    return in_maps


def kernel(**inputs) -> np.ndarray:
    from concourse.bass_utils import run_bass_kernel_spmd

    if "nc" not in _CACHE:
        _CACHE["nc"] = _build_program()
    nc = _CACHE["nc"]

    in_maps = _host_prep(inputs)
    res = run_bass_kernel_spmd(nc, in_maps, list(range(NCORES))).results

    out = np.empty((BATCH, SEQ, D_MODEL), dtype=np.float32)
    for c in range(NCORES):
        # batch 0: half-chunk layout (qc = c//2, half = c%2)
        t0 = (c // 2) * QC + (c % 2) * 2 * KT
        out[0, t0:t0 + 2 * KT, :] = res[c]["out"][0]
        # batch 1: quarter-chunk layout from the two split AllToAlls:
        # row block h holds tokens (2h + c//4)*512 + (c%4)*128
        for h in range(2):
            t1 = (2 * h + c // 4) * QC + (c % 4) * KT
            out[1, t1:t1 + KT, :] = res[c]["out"][1][h * KT:(h + 1) * KT]
    return out


# revision 65
# speedup vs baseline: 1.0181x; 1.0181x over previous
"""Causal multi-head self-attention with RoPE on 8 Trainium2 NeuronCores.

Sharding: tensor-parallel over heads — core c owns heads (2c, 2c+1) for BOTH
batch elements.  Feature dim lives on partitions, tokens on the free dim.

  phase A  (software-pipelined per 512-token chunk, per batch)
           qT/kT/vT = W @ x^T (f16 matmuls, K=1024 contraction);
           RoPE on qT/kT with HOST-precomputed cos/sin f16 tables
           (rot = x*C + swap(x)*S with the sign folded into S);
           vT transposed to (token, dim) 130-col k-tile layout with a ones
           column per head (denominator comes free out of the AV matmul).
  phase B  per (batch, q-chunk 512) — emitted with a 1-chunk skew so chunk
           t+1's projections hide chunk t's RoPE latency:
             logitsT (k-part, q-free) f16 = kT_h^T @ qT_h, 2 heads packed in
             one [128, 1024] psum; e = exp(logits/8) -> f16;
             AV runs transposed: ctx[q-part, 65] += e_blk^T @ [v | 1], one
             65-wide matmul per (head, q-tile 128, k-tile) — only q-tiles on
             or below the diagonal. Diagonal k-tiles get one 128x128
             triangular mask multiply per head.
           normalize by the ones-column (per-partition scalar on Pool),
           PE-transpose back to (dim, token), stage as f16.
  phase C  batch 0: one 8-core AllToAll (512 KB) exchanging half-chunks
           (core d gets tokens qc*512 + half*256 with qc = d//2, half =
           d%2), fully hidden under batch 1's compute.  Batch 1: TWO
           quarter-chunk AllToAlls (256 KB each) — the first (q-chunks 0-1)
           fires mid-attention and hides as well; only the second (q-chunks
           2-3, ~21 us) plus one 128-token projection sits in the serial
           tail.  Local out-projection with wo^T per arrival; dummy PE
           transposes keep the p-state warm across the last collective.
           Each core returns (2, 256, 1024); the host reassembles (batch 1
           uses the quarter-chunk slot mapping).
"""
import os
import sys

import numpy as np

for p in ("/opt/trn_rl_repo", "/root/.axon_site/_ro/trn_rl_repo"):
    if os.path.isdir(p) and p not in sys.path:
        sys.path.insert(0, p)

D_MODEL = 1024
NUM_HEADS = 16
D_K = 64
THETA = 10000.0
BATCH = 2
SEQ = 2048
NCORES = 8
H_PER_CORE = 2
DIMS = H_PER_CORE * D_K   # 128 ctx dims owned per core
QC = 512                  # q-chunk
KT = 128                  # k-tile
SCALE = 0.125             # 1/sqrt(d_k)

_CACHE = {}


def _build_program():
    import concourse.mybir as mybir
    import concourse.tile as tile
    from concourse import bacc
    from concourse.masks import make_identity, make_upper_triangular

    F32 = mybir.dt.float32
    F16 = mybir.dt.float16
    AFT = mybir.ActivationFunctionType

    nc = bacc.Bacc("TRN2", target_bir_lowering=False, debug=False,
                   num_devices=NCORES)

    xT_d = nc.declare_dram_parameter("xT", [D_MODEL, BATCH * SEQ], F16,
                                     isOutput=False)
    wqT_d = nc.declare_dram_parameter("wqT", [D_MODEL, DIMS], F16, isOutput=False)
    wkT_d = nc.declare_dram_parameter("wkT", [D_MODEL, DIMS], F16, isOutput=False)
    wvT_d = nc.declare_dram_parameter("wvT", [D_MODEL, DIMS], F16, isOutput=False)
    woT_d = nc.declare_dram_parameter("woT", [D_MODEL, D_MODEL], F16, isOutput=False)
    csC_d = nc.declare_dram_parameter("csC", [DIMS, BATCH * SEQ], F16,
                                      isOutput=False)
    csS_d = nc.declare_dram_parameter("csS", [DIMS, BATCH * SEQ], F16,
                                      isOutput=False)
    out_d = nc.declare_dram_parameter("out", [BATCH, 2 * KT, D_MODEL], F32,
                                      isOutput=True)
    DEBUG = bool(os.environ.get("K_DEBUG"))
    if DEBUG:
        dbg_qr = nc.declare_dram_parameter("dbg_qr", [DIMS, SEQ], F16,
                                           isOutput=True)
        dbg_kr = nc.declare_dram_parameter("dbg_kr", [DIMS, SEQ], F16,
                                           isOutput=True)
        dbg_vb = nc.declare_dram_parameter("dbg_vb", [128, 130 * (SEQ // KT)],
                                           F16, isOutput=True)
        dbg_st = nc.declare_dram_parameter("dbg_st", [NCORES, DIMS, 2 * KT],
                                           F16, isOutput=True)
        dbg_px = nc.declare_dram_parameter("dbg_px", [128, QC], F32,
                                           isOutput=True)
        dbg_et = nc.declare_dram_parameter("dbg_et", [2, 128, 2 * QC], F32,
                                           isOutput=True)

    NCH = SEQ // QC           # 4 chunks per batch
    NVT = QC // KT            # 4 k-tiles per chunk

    with tile.TileContext(nc) as tc:
        with tc.tile_pool(name="consts", bufs=1) as consts, \
             tc.tile_pool(name="qk", bufs=1) as qkp, \
             tc.tile_pool(name="vbufp", bufs=1) as vbufp, \
             tc.tile_pool(name="ps", bufs=1, space="PSUM") as ps, \
             tc.tile_pool(name="epool", bufs=17) as epool, \
             tc.tile_pool(name="rawp", bufs=3) as rawp, \
             tc.tile_pool(name="xtp", bufs=1) as xtp, \
             tc.tile_pool(name="ropep", bufs=3) as ropep, \
             tc.tile_pool(name="normp", bufs=6) as normp, \
             tc.tile_pool(name="stp", bufs=3) as stp, \
             tc.tile_pool(name="wp", bufs=1) as wp, \
             tc.tile_pool(name="outp", bufs=2) as outp, \
             tc.tile_pool(name="dram", bufs=1, space="DRAM") as dram:

            # ---------- constants ----------
            tri_f = consts.tile([KT, KT], F32)
            make_upper_triangular(nc, tri_f[:], val=1.0, diag=True)
            tri = consts.tile([KT, KT], F16)
            nc.vector.tensor_copy(tri, tri_f)
            ident = consts.tile([128, 128], F32)
            make_identity(nc, ident[:])
            ones16 = consts.tile([128, 16], F16)
            nc.vector.memset(ones16, 1.0)

            # cos/sin tables for both batches (host-precomputed); loaded
            # after the first x chunk (see below)
            csC = consts.tile([DIMS, BATCH * SEQ], F16, name="csC")
            csS = consts.tile([DIMS, BATCH * SEQ], F16, name="csS")

            a2a_in = [dram.tile([NCORES, DIMS, 2 * KT], F16, name=f"a2ain{b}")
                      for b in range(BATCH)]
            a2a_out = [dram.tile([NCORES, DIMS, 2 * KT], F16, name=f"a2aout{b}")
                       for b in range(BATCH)]
            a2a1_in = [dram.tile([NCORES, DIMS, KT], F16, name=f"a2b1in{h}")
                       for h in range(2)]
            a2a1_out = [dram.tile([NCORES, DIMS, KT], F16, name=f"a2b1out{h}")
                        for h in range(2)]

            # per-batch persistent tiles
            qR = {b: qkp.tile([DIMS, SEQ], F16, tag=f"qR{b}", name=f"qR{b}")
                  for b in range(BATCH)}
            kR = {b: qkp.tile([DIMS, SEQ], F16, tag=f"kR{b}", name=f"kR{b}")
                  for b in range(BATCH)}
            vbuf = {b: vbufp.tile([128, 130 * (SEQ // KT)], F16, tag=f"vb{b}",
                                  name=f"vbuf{b}")
                    for b in range(BATCH)}

            # projection weights
            w_sb = {}
            for nm, d in (("q", wqT_d), ("k", wkT_d), ("v", wvT_d)):
                wt = wp.tile([128, 8, DIMS], F16, tag=f"w{nm}", name=f"w{nm}")
                nc.sync.dma_start(
                    out=wt, in_=d[:].rearrange("(e p) c -> p e c", p=128))
                w_sb[nm] = [wt[:, k8, :] for k8 in range(8)]

            def emit_wo_loads():
                wo_sb = []
                for k8 in range(8):
                    t = wp.tile([128, D_MODEL], F16, tag=f"wo{k8}",
                                name=f"wo{k8}")
                    nc.sync.dma_start(out=t,
                                      in_=woT_d[k8 * 128:(k8 + 1) * 128, :])
                    wo_sb.append(t)
                return wo_sb

            # whole-x residency: 8 tiles of [128, 4096] f16, loaded in
            # dependency-epoch order so chunk 0 can start at once
            xt_all = [xtp.tile([128, BATCH * SEQ], F16, tag=f"xt{k8}",
                               name=f"xt{k8}")
                      for k8 in range(8)]

            def load_x(c0, c1):
                for k8 in range(8):
                    nc.sync.dma_start(
                        out=xt_all[k8][:, c0:c1],
                        in_=xT_d[k8 * 128:(k8 + 1) * 128, c0:c1])

            load_x(0, QC)
            nc.sync.dma_start(out=csC[:, 0:QC], in_=csC_d[:, 0:QC])
            nc.sync.dma_start(out=csS[:, 0:QC], in_=csS_d[:, 0:QC])
            load_x(QC, 2 * QC)
            nc.sync.dma_start(out=csC[:, QC:], in_=csC_d[:, QC:])
            nc.sync.dma_start(out=csS[:, QC:], in_=csS_d[:, QC:])
            load_x(2 * QC, SEQ)
            load_x(SEQ, SEQ + QC)
            load_x(SEQ + QC, 2 * SEQ)

            # ---------- phase A: one 512-token chunk ----------
            def emit_chunk(b, t):
                g0 = b * SEQ + t * QC          # global column offset
                c0 = t * QC                    # within-batch column offset
                # q,k projections -> one (dim, 2*QC) tile [q | k]
                rqk = rawp.tile([DIMS, 2 * QC], F16, tag="rawqk", name="rawqk")
                for ni, nm in enumerate(("q", "k")):
                    pp = ps.tile([128, QC], F32, tag="m1", bufs=2, name="pp")
                    for k8 in range(8):
                        nc.tensor.matmul(pp, w_sb[nm][k8],
                                         xt_all[k8][:, g0:g0 + QC],
                                         start=(k8 == 0), stop=(k8 == 7))
                    nc.vector.tensor_copy(rqk[:, ni * QC:(ni + 1) * QC], pp)

                # v: projected directly transposed, (token, dim) per 128-tile
                pv = ps.tile([128, QC], F32, tag="m1", bufs=2, name="pv")
                for i in range(NVT):
                    for k8 in range(8):
                        nc.tensor.matmul(
                            pv[:, i * KT:(i + 1) * KT],
                            xt_all[k8][:, g0 + i * KT:g0 + (i + 1) * KT],
                            w_sb["v"][k8],
                            start=(k8 == 0), stop=(k8 == 7),
                            skip_group_check=True)
                vb = vbuf[b]
                cview = vb[:, 130 * NVT * t:130 * NVT * (t + 1)]
                v_view = cview.rearrange("p (n c) -> p n c", c=130)
                for col in (64, 129):
                    nc.vector.tensor_copy(
                        v_view[:, :, col:col + 1].rearrange("p n c -> p (n c)"),
                        ones16[:, 0:NVT])
                dst = cview.rearrange("p (n h c) -> p n h c", h=2, c=65)[
                    :, :, :, 0:64]
                src = pv[:].rearrange("p (n h c) -> p n h c", h=2, c=64)
                nc.vector.tensor_copy(dst, src)

                # RoPE: rot = x*C + swap(x)*S   (sign baked into S rows)
                src = rqk[:].rearrange("(p two) n -> p two n", two=2)
                swp = ropep.tile([DIMS, 2 * QC], F16, tag="swap", name="swp")
                dstv = swp[:].rearrange("(p two) n -> p two n", two=2)
                nc.gpsimd.dma_start(out=dstv[:, 0, :], in_=src[:, 1, :])
                nc.gpsimd.dma_start(out=dstv[:, 1, :], in_=src[:, 0, :])
                for ni, nm in enumerate(("q", "k")):
                    nsl = slice(ni * QC, (ni + 1) * QC)
                    t1 = ropep.tile([DIMS, QC], F16, tag="t1", name="t1")
                    nc.vector.tensor_mul(t1, rqk[:, nsl], csC[:, g0:g0 + QC])
                    nc.vector.tensor_mul(swp[:, nsl], swp[:, nsl],
                                         csS[:, g0:g0 + QC])
                    dst = (qR if nm == "q" else kR)[b][:, c0:c0 + QC]
                    nc.vector.tensor_add(dst, t1, swp[:, nsl])

            # ---------- phase B: one q-chunk of attention ----------
            def emit_attn(b, qc):
                pctx = [ps.tile([128, QC], F32, tag=f"ctx{h}", bufs=1,
                                name=f"pctx{h}")
                        for h in range(H_PER_CORE)]
                nkt = NVT * qc + NVT
                ets = []
                for kt in range(nkt):
                    j = kt - NVT * qc            # >=0: diagonal band tile
                    q0 = 0 if j < 0 else KT * j
                    pl = ps.tile([128, 2 * QC], F32, tag="logit", bufs=2,
                                 name="pl")
                    for h in range(H_PER_CORE):
                        nc.tensor.matmul(
                            pl[:, h * QC + q0:(h + 1) * QC],
                            kR[b][64 * h:64 * (h + 1), kt * KT:(kt + 1) * KT],
                            qR[b][64 * h:64 * (h + 1),
                                  qc * QC + q0:(qc + 1) * QC],
                            start=True, stop=True)
                    et = epool.tile([128, 2 * QC], F16, tag="e", name="et")
                    if q0 == 0:
                        nc.scalar.activation(et, pl, AFT.Exp, scale=SCALE)
                    else:
                        ev = et[:].rearrange("p (h n) -> p h n", h=2)[
                            :, :, q0:QC]
                        pv = pl[:].rearrange("p (h n) -> p h n", h=2)[
                            :, :, q0:QC]
                        nc.scalar.activation(ev, pv, AFT.Exp, scale=SCALE)
                    if j >= 0:
                        for h in range(H_PER_CORE):
                            msl = slice(h * QC + q0, h * QC + q0 + KT)
                            nc.vector.tensor_mul(et[:, msl], et[:, msl], tri)
                    if DEBUG and b == 0 and qc == 0 and kt <= 1:
                        dt_ = normp.tile([128, 2 * QC], F32, tag="dbge",
                                         name="dbge", bufs=1)
                        nc.vector.tensor_copy(dt_, et)
                        nc.sync.dma_start(out=dbg_et[kt], in_=dt_)
                    ets.append(et)
                # AV, one accumulation group per (h, q-tile): only one open
                # group per psum bank at a time (PE constraint)
                for qt in range(NVT):
                    for kt in range(NVT * qc + qt + 1):
                        for h in range(H_PER_CORE):
                            vt = vbuf[b][:, 130 * kt + 65 * h:
                                         130 * kt + 65 * h + 65]
                            nc.tensor.matmul(
                                pctx[h][:, qt * KT:qt * KT + 65],
                                ets[kt][:, h * QC + qt * KT:
                                        h * QC + (qt + 1) * KT],
                                vt,
                                start=(kt == 0),
                                stop=(kt == NVT * qc + qt),
                                skip_group_check=True)

                if DEBUG and b == 0 and qc == 0:
                    dpx = normp.tile([128, QC], F32, tag="dbgp", name="dbgp",
                                     bufs=1)
                    nc.vector.tensor_copy(dpx, pctx[0][:])
                    nc.sync.dma_start(out=dbg_px[:], in_=dpx)

                # normalize, transpose to (dim, token), stage for the A2A
                recips = []
                for h in range(H_PER_CORE):
                    rc = normp.tile([128, NVT], F32, tag="rc", name="rc")
                    den = pctx[h][:].rearrange("p (n c) -> p n c", c=KT)[
                        :, :, 64:65].rearrange("p n c -> p (n c)")
                    nc.vector.reciprocal(rc, den)
                    recips.append(rc)
                stg = stp.tile([128, QC], F16, tag="stage", name="stage")
                for qt in range(NVT):
                    tp = ps.tile([128, QC], F32, tag="m1", bufs=2, name="tp")
                    nr = normp.tile([128, 128], F32, tag="nr", name="nr")
                    for h in range(H_PER_CORE):
                        nc.vector.tensor_scalar_mul(
                            nr[:, 64 * h:64 * (h + 1)],
                            pctx[h][:, qt * KT:qt * KT + 64],
                            recips[h][:, qt:qt + 1])
                    nc.tensor.transpose(tp[:, 0:128], nr, ident[:])
                    nc.vector.tensor_copy(stg[:, qt * KT:(qt + 1) * KT],
                                          tp[:, 0:128])
                if b == 0:
                    for half in range(2):
                        nc.sync.dma_start(
                            out=a2a_in[0][2 * qc + half],
                            in_=stg[:, half * 2 * KT:(half + 1) * 2 * KT])
                else:
                    # batch 1 exchanges in two quarter-chunk AllToAlls; slot
                    # d of half h = tokens ((2h + qc%2)*512 + (d%4)*128)
                    dst = a2a1_in[qc // 2]
                    for qt in range(NVT):
                        nc.sync.dma_start(
                            out=dst[4 * (qc % 2) + qt],
                            in_=stg[:, qt * KT:(qt + 1) * KT])

            def emit_a2a(b):
                nc.gpsimd.collective_compute(
                    "AllToAll", mybir.AluOpType.bypass,
                    replica_groups=[list(range(NCORES))],
                    ins=[a2a_in[b].opt()], outs=[a2a_out[b].opt()],
                )

            def emit_a2a1(half):
                nc.gpsimd.collective_compute(
                    "AllToAll", mybir.AluOpType.bypass,
                    replica_groups=[list(range(NCORES))],
                    ins=[a2a1_in[half].opt()], outs=[a2a1_out[half].opt()],
                )

            def emit_woproj1(half, wo_sb):
                cm = wp.tile([DIMS, NCORES, KT], F16, tag=f"cm1{half}",
                             name=f"cm1{half}")
                nc.sync.dma_start(
                    out=cm, in_=a2a1_out[half][:].rearrange("e p c -> p e c"))
                ot = outp.tile([128, D_MODEL], F32, tag="out", name="ot")
                for nn in range(2):
                    po = ps.tile([128, QC], F32, tag="m1", bufs=2, name="po")
                    for i in range(NCORES):
                        nc.tensor.matmul(
                            po, cm[:, i, :], wo_sb[i][:, nn * QC:(nn + 1) * QC],
                            start=(i == 0), stop=(i == NCORES - 1))
                    nc.scalar.copy(ot[:, nn * QC:(nn + 1) * QC], po)
                    nc.sync.dma_start(
                        out=out_d[1, half * KT:(half + 1) * KT,
                                  nn * QC:(nn + 1) * QC],
                        in_=ot[:, nn * QC:(nn + 1) * QC])

            def emit_woproj(b, wo_sb):
                cm = wp.tile([DIMS, NCORES, 2 * KT], F16, tag=f"cm{b}",
                             name=f"cm{b}")
                nc.sync.dma_start(
                    out=cm, in_=a2a_out[b][:].rearrange("e p c -> p e c"))
                for mt in range(2):
                    ot = outp.tile([128, D_MODEL], F32, tag="out", name="ot")
                    for nn in range(2):
                        po = ps.tile([128, QC], F32, tag="m1", bufs=2,
                                     name="po")
                        for i in range(NCORES):
                            nc.tensor.matmul(
                                po, cm[:, i, mt * KT:(mt + 1) * KT],
                                wo_sb[i][:, nn * QC:(nn + 1) * QC],
                                start=(i == 0), stop=(i == NCORES - 1))
                        nc.scalar.copy(ot[:, nn * QC:(nn + 1) * QC], po)
                        nc.sync.dma_start(
                            out=out_d[b, mt * KT:(mt + 1) * KT,
                                      nn * QC:(nn + 1) * QC],
                            in_=ot[:, nn * QC:(nn + 1) * QC])

            # ---------- emission schedule ----------
            emit_chunk(0, 0)
            wo_sb = emit_wo_loads()
            emit_chunk(0, 1)
            emit_attn(0, 0)
            emit_chunk(0, 2)
            emit_attn(0, 1)
            emit_chunk(0, 3)
            emit_attn(0, 2)
            emit_chunk(1, 0)
            emit_attn(0, 3)
            emit_chunk(1, 1)
            emit_attn(1, 0)
            emit_chunk(1, 2)
            emit_a2a(0)
            emit_attn(1, 1)
            emit_chunk(1, 3)
            emit_a2a1(0)
            emit_attn(1, 2)
            emit_attn(1, 3)
            emit_a2a1(1)
            emit_woproj(0, wo_sb)
            emit_woproj1(0, wo_sb)
            # keep the PE p-state warm while the last AllToAll is in
            # flight so the final projection runs at full clock
            for _ in range(140):
                dpl = ps.tile([128, QC], F32, tag="m1", bufs=2, name="dpl")
                nc.tensor.transpose(dpl[:, 0:128], ident, ident[:])
            emit_woproj1(1, wo_sb)
            if DEBUG:
                nc.sync.dma_start(out=dbg_qr[:], in_=qR[0])
                nc.sync.dma_start(out=dbg_kr[:], in_=kR[0])
                nc.sync.dma_start(out=dbg_vb[:], in_=vbuf[0])
                cm0 = wp.tile([DIMS, NCORES, 2 * KT], F16, tag="cm0",
                              name="cmdbg")
                nc.sync.dma_start(
                    out=cm0, in_=a2a_out[0][:].rearrange("e p c -> p e c"))
                nc.sync.dma_start(
                    out=dbg_st[:].rearrange("e p c -> p e c"), in_=cm0)

    nc.compile()
    return nc


def _host_prep(inputs):
    x = np.asarray(inputs["in_features"], dtype=np.float32)
    tp = np.asarray(inputs["token_positions"], dtype=np.int32)
    wq = np.asarray(inputs["wq"], dtype=np.float32)
    wk = np.asarray(inputs["wk"], dtype=np.float32)
    wv = np.asarray(inputs["wv"], dtype=np.float32)
    wo = np.asarray(inputs["wo"], dtype=np.float32)

    xT = np.ascontiguousarray(
        np.concatenate([x[b].T for b in range(BATCH)], axis=1)).astype(np.float16)
    woT = np.ascontiguousarray(wo.T).astype(np.float16)

    # cos/sin tables, (dim row, batch*token col); sign baked into S so that
    # rot = x*C + swap(x)*S
    half = D_K // 2
    inv_freq = 1.0 / (THETA ** (2.0 * np.arange(half) / D_K))     # (32,)
    ang = tp.astype(np.float64)[:, :, None] * inv_freq[None, None, :]
    cos = np.cos(ang)                                             # (B, S, 32)
    sin = np.sin(ang)
    rows = np.arange(DIMS)
    j = (rows % D_K) // 2                                         # freq index
    sign = np.where(rows % 2 == 0, -1.0, 1.0)
    csC = np.empty((DIMS, BATCH * SEQ), dtype=np.float16)
    csS = np.empty((DIMS, BATCH * SEQ), dtype=np.float16)
    for b in range(BATCH):
        csC[:, b * SEQ:(b + 1) * SEQ] = cos[b][:, j].T
        csS[:, b * SEQ:(b + 1) * SEQ] = (sin[b][:, j] * sign[None, :]).T

    in_maps = []
    for c in range(NCORES):
        rsl = slice(DIMS * c, DIMS * (c + 1))
        in_maps.append({
            "xT": xT,
            "wqT": np.ascontiguousarray(wq[rsl].T).astype(np.float16),
            "wkT": np.ascontiguousarray(wk[rsl].T).astype(np.float16),
            "wvT": np.ascontiguousarray(wv[rsl].T).astype(np.float16),
            "woT": woT,
            "csC": csC,
            "csS": csS,
        })
    return in_maps


def kernel(**inputs) -> np.ndarray:
    from concourse.bass_utils import run_bass_kernel_spmd

    if "nc" not in _CACHE:
        _CACHE["nc"] = _build_program()
    nc = _CACHE["nc"]

    in_maps = _host_prep(inputs)
    res = run_bass_kernel_spmd(nc, in_maps, list(range(NCORES))).results

    out = np.empty((BATCH, SEQ, D_MODEL), dtype=np.float32)
    for c in range(NCORES):
        # batch 0: half-chunk layout (qc = c//2, half = c%2)
        t0 = (c // 2) * QC + (c % 2) * 2 * KT
        out[0, t0:t0 + 2 * KT, :] = res[c]["out"][0]
        # batch 1: quarter-chunk layout from the two split AllToAlls:
        # row block h holds tokens (2h + c//4)*512 + (c%4)*128
        for h in range(2):
            t1 = (2 * h + c // 4) * QC + (c % 4) * KT
            out[1, t1:t1 + KT, :] = res[c]["out"][1][h * KT:(h + 1) * KT]
    return out
